# revision 9
# baseline (speedup 1.0000x reference)
"""Trainium2 Bass kernel for nn_MemoryCell (scatter_memory), v4.

Full-input contract: kernel(**inputs) takes the complete (unsharded) numpy
inputs and returns the full [NB*B, H] output.

Math (B == H == 1024, NB == 5, T == 128):
    enc  = features[:, 0, :]                         # [B, H] - only slice used
    h    = states.reshape(NB, H)
    gate = sigmoid(enc @ (h + keys).T)               # [B, NB]
    pre  = (h @ Uw.T + keys @ Vw.T)[:, None, :] + (enc @ Ww.T)[None, :, :]
    cand = where(pre >= 0, pre, prelu_a * pre)
    new[i, b, j] = h[i, j] + gate[j, i] * cand[i, b, j]   # B==H broadcast quirk
    out  = sign(new) with exact zeros -> +1, reshaped [NB*B, H]

Because gate > 0 and (for prelu slope a > 0) new is monotone in ew =
enc @ Ww.T, each output element is a pure threshold test:

    sign(new[i, b, j]) = +1  iff  ew[j, b] + nthr[j, i] >= 0
    nthr = huv + (h / s) * (1 + exp(-z)),  s = a if h > 0 else 1

nthr is a tiny [H, NB] tensor: the host computes it exactly (float64) from
the small operands.  The device only does the big work: stream enc (fp16)
+ the j-shard of Ww, run the [128 x 1024 x 1024] matmul, and apply one
compare per (block, half) - ACT Sign(ew + bias) / DVE is_ge - writing int8.

Sharding: j (feature) axis split into 8 shards of 128, one per core.
Per-core HBM: 2.37 MB in, 0.65 MB out.

Hardware notes baked into the structure (from perfetto traces):
  * PSUM bank reads serialize across engines per instruction, so the ACT
    and DVE tail lanes get their OWN PSUM banks: the ew matmul is emitted
    as two series per half (cols 0:288 -> ACT bank, 288:512 -> DVE bank).
  * ACT and DVE lanes write separate SBUF output tiles (a shared tile
    serializes the writers through the framework's WAW ordering).
  * Each dma_start costs ~650ns on the shared HWDGE descriptor engine,
    and concurrently active DMA rings share HBM bandwidth round-robin;
    4 dummy dma_starts between the encA and encB issues stagger the two
    halves so encA (and the half-A compute) completes ~2.5us earlier.
  * ~32 PE warm-up transposes bridge the DMA wait so the real matmuls
    run at full clock (PE needs ~3us of continuous activity).
Measured 109 sign flips vs the 524-flip (2e-2 rel err) budget.
"""

import numpy as np

H = 1024
NB = 5
B = 1024
NCORES = 8
JS = H // NCORES          # 128 feature columns per core
KC = H // 128             # 8 contraction chunks
HB = 512                  # b half width (one PSUM bank of fp32)
AW = 288                  # tail columns on ACT per half (rest on DVE)
DW = HB - AW
WARMUP = 32

_NC_CACHE = {}


def _build_nc():
    from concourse import bacc, mybir
    import concourse.tile as tile
    from concourse.masks import make_identity

    f32 = mybir.dt.float32
    f16 = mybir.dt.float16
    i8 = mybir.dt.int8
    AF = mybir.ActivationFunctionType
    ALU = mybir.AluOpType

    nc = bacc.Bacc("TRN2", debug=False, num_devices=NCORES)

    wt_d = nc.dram_tensor("wt", [128, KC, 128], f16, kind="ExternalInput").ap()
    thr_d = nc.dram_tensor("thr", [128, 16], f32, kind="ExternalInput").ap()
    encA_d = nc.dram_tensor("encA", [128, KC, HB], f16, kind="ExternalInput").ap()
    encB_d = nc.dram_tensor("encB", [128, KC, HB], f16, kind="ExternalInput").ap()
    dum_d = nc.dram_tensor("dum", [128, 4], f16, kind="ExternalInput").ap()
    oa_d = nc.dram_tensor("oa", [128, 2, NB, AW], i8, kind="ExternalOutput").ap()
    od_d = nc.dram_tensor("od", [128, 2, NB, DW], i8, kind="ExternalOutput").ap()

    with tile.TileContext(nc) as tc:
        with (
            tc.tile_pool(name="res", bufs=1) as res,
            tc.tile_pool(name="ps", bufs=1, space="PSUM") as ps,
        ):
            # ---- input DMAs on SyncE; 4 dummy issues delay encB's ring ----
            wt = res.tile([128, KC, 128], f16, name="wt")
            thr = res.tile([128, 16], f32, name="thr")
            encA = res.tile([128, KC, HB], f16, name="encA")
            encB = res.tile([128, KC, HB], f16, name="encB")
            # big tensors ship as two partition-sliced rings each: concurrent
            # rings pipeline DMA descriptor setup (a single ring is capped at
            # ~230 GB/s by per-descriptor overhead)
            nc.sync.dma_start(wt, wt_d)
            nc.sync.dma_start(thr, thr_d)
            nc.sync.dma_start(encA[0:64], encA_d[0:64])
            nc.sync.dma_start(encA[64:128], encA_d[64:128])
            for i in range(2):
                dt_ = res.tile([128, 1], f16, name=f"dum{i}")
                nc.sync.dma_start(dt_, dum_d[:, i:i + 1])
            nc.sync.dma_start(encB[0:64], encB_d[0:64])
            nc.sync.dma_start(encB[64:128], encB_d[64:128])

            # ---- PSUM: full-bank tiles so each tail lane owns its bank ----
            pwarm = ps.tile([128, 128], f32, name="pwarm")
            pAL = ps.tile([128, HB], f32, name="pAL")
            pAR = ps.tile([128, HB], f32, name="pAR")
            pBL = ps.tile([128, HB], f32, name="pBL")
            pBR = ps.tile([128, HB], f32, name="pBR")

            # PE warm-up: dummy transposes bridge the DMA wait (PE needs
            # ~3us of continuous activity to reach full clock)
            identity = res.tile([128, 128], f32, name="identity")
            make_identity(nc, identity)
            for _ in range(WARMUP):
                nc.tensor.transpose(pwarm, identity, identity)

            # ew[j, b] = sum_k Ww[j,k] enc[b,k]; per half: L-series (ACT's
            # bank, cols 0:AW) + R-series (DVE's bank, cols AW:HB)
            for pl, pr, et in ((pAL, pAR, encA), (pBL, pBR, encB)):
                for k in range(KC):
                    nc.tensor.matmul(pl[:, 0:AW], lhsT=wt[:, k, :],
                                     rhs=et[:, k, 0:AW],
                                     start=(k == 0), stop=(k == KC - 1))
                    nc.tensor.matmul(pr[:, 0:DW], lhsT=wt[:, k, :],
                                     rhs=et[:, k, AW:HB],
                                     start=(k == 0), stop=(k == KC - 1))

            # ---- tail: ACT Sign(ew + nthr_i) {-1,0,1} (host: >= 0 -> +1);
            #            DVE (ew >= tpos_i) {1,0}     (host: > 0  -> +1)
            o_act = res.tile([128, 2, NB, AW], i8, name="o_act")
            o_dve = res.tile([128, 2, NB, DW], i8, name="o_dve")
            for hf, pl, pr in ((0, pAL, pAR), (1, pBL, pBR)):
                for i in range(NB):
                    nc.scalar.activation(o_act[:, hf, i, :], pl[:, 0:AW],
                                         AF.Sign, bias=thr[:, i:i + 1])
                    nc.vector.tensor_scalar(o_dve[:, hf, i, :], pr[:, 0:DW],
                                            thr[:, 5 + i:6 + i], None,
                                            ALU.is_ge)
                nc.sync.dma_start(oa_d[:, hf], o_act[:, hf])
                nc.sync.dma_start(od_d[:, hf], o_dve[:, hf])

    nc.compile()
    return nc


def _get_nc():
    nc = _NC_CACHE.get("nc")
    if nc is None:
        nc = _build_nc()
        _NC_CACHE["nc"] = nc
    return nc


def _f16(a):
    return np.ascontiguousarray(a, dtype=np.float16)


def _chunkT(mat):
    # [H(k), F] -> [128, KC, F]: partition p holds k-chunk rows k*128+p
    F = mat.shape[1]
    return np.ascontiguousarray(mat.reshape(KC, 128, F).transpose(1, 0, 2))


def _numpy_fallback(enc, h, keys, Uw, Vw, Ww, prelu_a):
    gate = 1.0 / (1.0 + np.exp(-(enc @ (h + keys).T)))
    pre = (h @ Uw.T + keys @ Vw.T)[:, None, :] + (enc @ Ww.T)[None, :, :]
    cand = np.where(pre >= 0, pre, prelu_a * pre)
    new = h[:, None, :] + gate.T[:, None, :] * cand
    new = np.where(new == 0, np.float32(0.1), new)
    new = np.sign(new).astype(np.float32)
    return new.reshape(NB * B, H)


def kernel(features, states, Uw, Vw, Ww, keys, prelu_a):
    from concourse import bass_utils
    import os

    features = np.asarray(features)
    states = np.asarray(states, dtype=np.float32)
    Uw = np.asarray(Uw, dtype=np.float32)
    Vw = np.asarray(Vw, dtype=np.float32)
    Ww = np.asarray(Ww, dtype=np.float32)
    keys = np.asarray(keys, dtype=np.float32)
    prelu_a = np.asarray(prelu_a, dtype=np.float32)

    enc = np.ascontiguousarray(features[:, 0, :], dtype=np.float32)  # [B, H]
    h = states.reshape(NB, H)

    if np.any(prelu_a <= 0):
        # new is not monotone in ew for a <= 0; never hit in practice
        return _numpy_fallback(enc, h, keys, Uw, Vw, Ww, prelu_a)
    nc = _get_nc()

    # exact thresholds (float64) from the small operands
    e64 = enc.astype(np.float64)
    h64 = h.astype(np.float64)
    k64 = keys.astype(np.float64)
    z = e64 @ (h64 + k64).T                                   # [j, i]
    huv = Uw.astype(np.float64) @ h64.T + Vw.astype(np.float64) @ k64.T
    s = np.where(h64.T > 0, prelu_a.astype(np.float64)[:, None], 1.0)
    with np.errstate(over='ignore'):
        hos = h64.T / s
        nthr = huv + hos * (1.0 + np.exp(-z))
    nthr = np.clip(nthr, -1e30, 1e30).astype(np.float32)      # [H(j), NB]

    # enc.T fp16-single, chunked [128, KC, B]; shared by all cores
    e3 = _chunkT(_f16(enc.T))
    encA = np.ascontiguousarray(e3[:, :, 0:HB])
    encB = np.ascontiguousarray(e3[:, :, HB:B])
    dum = np.zeros((128, 4), dtype=np.float16)

    in_maps = []
    for c in range(NCORES):
        js = slice(c * JS, (c + 1) * JS)
        thr = np.zeros((128, 16), dtype=np.float32)
        thr[:, 0:5] = nthr[js]
        thr[:, 5:10] = -nthr[js]
        in_maps.append({
            "wt": _chunkT(_f16(Ww[js].T)),
            "thr": thr,
            "encA": encA,
            "encB": encB,
            "dum": dum,
        })

    trace = bool(int(os.environ.get("KERNEL_TRACE", "0")))
    res = bass_utils.run_bass_kernel_spmd(
        nc, in_maps, core_ids=list(range(NCORES)), trace=trace)
    kernel.last_result = res

    one = np.float32(1.0)
    neg = np.float32(-1.0)
    full = np.empty((NB, B, H), dtype=np.float32)
    ok = np.empty((NB, HB, 128), dtype=np.float32)
    for c in range(NCORES):
        oa = res.results[c]["oa"]                  # [128, 2, NB, AW] int8
        od = res.results[c]["od"]                  # [128, 2, NB, DW] int8
        for hf in range(2):
            a = oa[:, hf].transpose(1, 2, 0)       # [NB, AW, 128]
            d = od[:, hf].transpose(1, 2, 0)       # [NB, DW, 128]
            ok[:, 0:AW] = np.where(a >= 0, one, neg)
            ok[:, AW:HB] = np.where(d > 0, one, neg)
            full[:, hf * HB:(hf + 1) * HB, c * JS:(c + 1) * JS] = ok
    return full.reshape(NB * B, H)


# revision 11
# speedup vs baseline: 1.2251x; 1.2251x over previous
"""Trainium2 Bass kernel for nn_MemoryCell (scatter_memory), v4.

Full-input contract: kernel(**inputs) takes the complete (unsharded) numpy
inputs and returns the full [NB*B, H] output.

Math (B == H == 1024, NB == 5, T == 128):
    enc  = features[:, 0, :]                         # [B, H] - only slice used
    h    = states.reshape(NB, H)
    gate = sigmoid(enc @ (h + keys).T)               # [B, NB]
    pre  = (h @ Uw.T + keys @ Vw.T)[:, None, :] + (enc @ Ww.T)[None, :, :]
    cand = where(pre >= 0, pre, prelu_a * pre)
    new[i, b, j] = h[i, j] + gate[j, i] * cand[i, b, j]   # B==H broadcast quirk
    out  = sign(new) with exact zeros -> +1, reshaped [NB*B, H]

Because gate > 0 and (for prelu slope a > 0) new is monotone in ew =
enc @ Ww.T, each output element is a pure threshold test:

    sign(new[i, b, j]) = +1  iff  ew[j, b] + nthr[j, i] >= 0
    nthr = huv + (h / s) * (1 + exp(-z)),  s = a if h > 0 else 1

nthr is a tiny [H, NB] tensor: the host computes it exactly (float64) from
the small operands.  The device only does the big work: stream enc (fp16)
+ the j-shard of Ww, run the [128 x 1024 x 1024] matmul, and apply one
compare per (block, half) - ACT Sign(ew + bias) / DVE is_ge - writing int8.

Sharding: j (feature) axis split into 8 shards of 128, one per core.
Per-core HBM: 2.37 MB in, 0.65 MB out.

Hardware notes baked into the structure (from perfetto traces):
  * PSUM bank reads serialize across engines per instruction, so the ACT
    and DVE tail lanes get their OWN PSUM banks: the ew matmul is emitted
    as two series per half (cols 0:288 -> ACT bank, 288:512 -> DVE bank).
  * ACT and DVE lanes write separate SBUF output tiles (a shared tile
    serializes the writers through the framework's WAW ordering).
  * Each dma_start costs ~650ns on the shared HWDGE descriptor engine,
    and concurrently active DMA rings share HBM bandwidth round-robin;
    4 dummy dma_starts between the encA and encB issues stagger the two
    halves so encA (and the half-A compute) completes ~2.5us earlier.
  * ~32 PE warm-up transposes bridge the DMA wait so the real matmuls
    run at full clock (PE needs ~3us of continuous activity).
Measured 109 sign flips vs the 524-flip (2e-2 rel err) budget.
"""

import numpy as np

H = 1024
NB = 5
B = 1024
NCORES = 8
JS = H // NCORES          # 128 feature columns per core
KC = H // 128             # 8 contraction chunks
HB = 512                  # b half width (one PSUM bank of fp32)
AW = 256                  # tail columns on ACT per half (rest on DVE)
DW = HB - AW
WARMUP = 32

_NC_CACHE = {}


def _build_nc():
    from concourse import bacc, mybir
    import concourse.tile as tile
    from concourse.masks import make_identity

    f32 = mybir.dt.float32
    f16 = mybir.dt.float16
    i8 = mybir.dt.int8
    AF = mybir.ActivationFunctionType
    ALU = mybir.AluOpType

    nc = bacc.Bacc("TRN2", debug=False, num_devices=NCORES)

    wt_d = nc.dram_tensor("wt", [128, KC, 128], f16, kind="ExternalInput").ap()
    thr_d = nc.dram_tensor("thr", [128, 16], f32, kind="ExternalInput").ap()
    enc_d = [nc.dram_tensor(f"enc{q}", [128, KC, 256], f16,
                            kind="ExternalInput").ap() for q in range(4)]
    dum_d = nc.dram_tensor("dum", [128, 4], f16, kind="ExternalInput").ap()
    oa_d = nc.dram_tensor("oa", [128, 2, NB, AW], i8, kind="ExternalOutput").ap()
    od_d = nc.dram_tensor("od", [128, 2, NB, DW], i8, kind="ExternalOutput").ap()

    with tile.TileContext(nc) as tc:
        with (
            tc.tile_pool(name="res", bufs=1) as res,
            tc.tile_pool(name="ps", bufs=1, space="PSUM") as ps,
        ):
            # ---- input DMAs on SyncE; 4 dummy issues delay encB's ring ----
            wt = res.tile([128, KC, 128], f16, name="wt")
            thr = res.tile([128, 16], f32, name="thr")
            enc = [res.tile([128, KC, 256], f16, name=f"enc{q}")
                   for q in range(4)]
            # enc ships as 4 b-quarter rings (4KB descriptors): concurrent
            # rings keep the DMA descriptor pipelines fed; the A-half rings
            # go first (+2 dummy issues) so half A lands early
            nc.sync.dma_start(wt, wt_d)
            nc.sync.dma_start(thr, thr_d)
            nc.sync.dma_start(enc[0], enc_d[0])
            nc.sync.dma_start(enc[1], enc_d[1])
            for i in range(2):
                dt_ = res.tile([128, 1], f16, name=f"dum{i}")
                nc.sync.dma_start(dt_, dum_d[:, i:i + 1])
            nc.sync.dma_start(enc[2], enc_d[2])
            nc.sync.dma_start(enc[3], enc_d[3])

            # ---- PSUM: full-bank tiles so each tail lane owns its bank ----
            pwarm = ps.tile([128, 128], f32, name="pwarm")
            pAL = ps.tile([128, HB], f32, name="pAL")
            pAR = ps.tile([128, HB], f32, name="pAR")
            pBL = ps.tile([128, HB], f32, name="pBL")
            pBR = ps.tile([128, HB], f32, name="pBR")

            # PE warm-up: dummy transposes bridge the DMA wait (PE needs
            # ~3us of continuous activity to reach full clock)
            identity = res.tile([128, 128], f32, name="identity")
            make_identity(nc, identity)
            for _ in range(WARMUP):
                nc.tensor.transpose(pwarm, identity, identity)

            # ew[j, b] = sum_k Ww[j,k] enc[b,k]; one series per quarter,
            # each into its own PSUM bank (ACT reads L banks, DVE R banks)
            for pq, eq in ((pAL, enc[0]), (pAR, enc[1]),
                           (pBL, enc[2]), (pBR, enc[3])):
                for k in range(KC):
                    nc.tensor.matmul(pq[:, 0:256], lhsT=wt[:, k, :],
                                     rhs=eq[:, k, :],
                                     start=(k == 0), stop=(k == KC - 1))

            # ---- tail: ACT Sign(ew + nthr_i) {-1,0,1} (host: >= 0 -> +1);
            #            DVE (ew >= tpos_i) {1,0}     (host: > 0  -> +1)
            o_act = res.tile([128, 2, NB, AW], i8, name="o_act")
            o_dve = res.tile([128, 2, NB, DW], i8, name="o_dve")
            for hf, pl, pr in ((0, pAL, pAR), (1, pBL, pBR)):
                for i in range(NB):
                    nc.scalar.activation(o_act[:, hf, i, :], pl[:, 0:AW],
                                         AF.Sign, bias=thr[:, i:i + 1])
                    nc.vector.tensor_scalar(o_dve[:, hf, i, :], pr[:, 0:DW],
                                            thr[:, 5 + i:6 + i], None,
                                            ALU.is_ge)
                nc.sync.dma_start(oa_d[:, hf], o_act[:, hf])
                nc.sync.dma_start(od_d[:, hf], o_dve[:, hf])

    nc.compile()
    return nc


def _get_nc():
    nc = _NC_CACHE.get("nc")
    if nc is None:
        nc = _build_nc()
        _NC_CACHE["nc"] = nc
    return nc


def _f16(a):
    return np.ascontiguousarray(a, dtype=np.float16)


def _chunkT(mat):
    # [H(k), F] -> [128, KC, F]: partition p holds k-chunk rows k*128+p
    F = mat.shape[1]
    return np.ascontiguousarray(mat.reshape(KC, 128, F).transpose(1, 0, 2))


def _numpy_fallback(enc, h, keys, Uw, Vw, Ww, prelu_a):
    gate = 1.0 / (1.0 + np.exp(-(enc @ (h + keys).T)))
    pre = (h @ Uw.T + keys @ Vw.T)[:, None, :] + (enc @ Ww.T)[None, :, :]
    cand = np.where(pre >= 0, pre, prelu_a * pre)
    new = h[:, None, :] + gate.T[:, None, :] * cand
    new = np.where(new == 0, np.float32(0.1), new)
    new = np.sign(new).astype(np.float32)
    return new.reshape(NB * B, H)


def kernel(features, states, Uw, Vw, Ww, keys, prelu_a):
    from concourse import bass_utils
    import os

    features = np.asarray(features)
    states = np.asarray(states, dtype=np.float32)
    Uw = np.asarray(Uw, dtype=np.float32)
    Vw = np.asarray(Vw, dtype=np.float32)
    Ww = np.asarray(Ww, dtype=np.float32)
    keys = np.asarray(keys, dtype=np.float32)
    prelu_a = np.asarray(prelu_a, dtype=np.float32)

    enc = np.ascontiguousarray(features[:, 0, :], dtype=np.float32)  # [B, H]
    h = states.reshape(NB, H)

    if np.any(prelu_a <= 0):
        # new is not monotone in ew for a <= 0; never hit in practice
        return _numpy_fallback(enc, h, keys, Uw, Vw, Ww, prelu_a)
    nc = _get_nc()

    # exact thresholds (float64) from the small operands
    e64 = enc.astype(np.float64)
    h64 = h.astype(np.float64)
    k64 = keys.astype(np.float64)
    z = e64 @ (h64 + k64).T                                   # [j, i]
    huv = Uw.astype(np.float64) @ h64.T + Vw.astype(np.float64) @ k64.T
    s = np.where(h64.T > 0, prelu_a.astype(np.float64)[:, None], 1.0)
    with np.errstate(over='ignore'):
        hos = h64.T / s
        nthr = huv + hos * (1.0 + np.exp(-z))
    nthr = np.clip(nthr, -1e30, 1e30).astype(np.float32)      # [H(j), NB]

    # enc.T fp16-single, chunked [128, KC, B]; shared by all cores
    e3 = _chunkT(_f16(enc.T))
    encq = {f"enc{q}": np.ascontiguousarray(e3[:, :, q * 256:(q + 1) * 256])
            for q in range(4)}
    dum = np.zeros((128, 4), dtype=np.float16)

    in_maps = []
    for c in range(NCORES):
        js = slice(c * JS, (c + 1) * JS)
        thr = np.zeros((128, 16), dtype=np.float32)
        thr[:, 0:5] = nthr[js]
        thr[:, 5:10] = -nthr[js]
        in_maps.append({
            "wt": _chunkT(_f16(Ww[js].T)),
            "thr": thr,
            "dum": dum,
            **encq,
        })

    trace = bool(int(os.environ.get("KERNEL_TRACE", "0")))
    res = bass_utils.run_bass_kernel_spmd(
        nc, in_maps, core_ids=list(range(NCORES)), trace=trace)
    kernel.last_result = res

    one = np.float32(1.0)
    neg = np.float32(-1.0)
    full = np.empty((NB, B, H), dtype=np.float32)
    ok = np.empty((NB, HB, 128), dtype=np.float32)
    for c in range(NCORES):
        oa = res.results[c]["oa"]                  # [128, 2, NB, AW] int8
        od = res.results[c]["od"]                  # [128, 2, NB, DW] int8
        for hf in range(2):
            a = oa[:, hf].transpose(1, 2, 0)       # [NB, AW, 128]
            d = od[:, hf].transpose(1, 2, 0)       # [NB, DW, 128]
            ok[:, 0:AW] = np.where(a >= 0, one, neg)
            ok[:, AW:HB] = np.where(d > 0, one, neg)
            full[:, hf * HB:(hf + 1) * HB, c * JS:(c + 1) * JS] = ok
    return full.reshape(NB * B, H)
